# revision 5
# baseline (speedup 1.0000x reference)
"""Trainium2 Bass kernel for a BasicTransformerBlock (B=2, S=2048, H=768, FF=3072, NH=12).

Sharding: core c handles batch b=c//4, sequence quarter q=c%4 (512 tokens).
Each core redundantly computes LN1 + K/V projections for its batch's full
2048 tokens (no collectives); Q/attention/Wo/FFN only for its own 512 tokens.

v3 over the 306us v2 baseline (v2 was ACT-bound in attention: 96 exp
ACTIVATEs ~113us serial on the scalar engine while tensor idled):
- EXP4_ANT custom DVE op: exp(4u) = (1 + c1 u + c2 u^2 + c3 u^3)^4 (deg-3
  Horner + two SQUARE stages = exactly the DVE's 8 ALU stages; shifts are
  dead on DVE so no exponent-stuffing).  Scores are pre-scaled by 1/32
  (folded into the K projection weights) so the psum value IS u; max rel
  err 0.55% over the observed score range, invisible under the fp8 e4m3
  prob quantization (5.9%).  Attention exp is split per (j,hp): half 0 on
  ACT (exp, scale=4), half 1 on DVE (EXP4) -> both engines ~60us, tensor
  becomes the attention bottleneck.
- DMA restructure: latbf tiles land first ([NTT,P,FC,TQ] host layout, 6KB
  contiguous lines), weights follow in need-order; wo/resid1/w2 deferred to
  phase-2 issue points (needed >100us later).
- Wo loop mc-outer/hp-inner: wo psum chunk mc completes after 6 MMs, its
  residual+LN2-stats chain overlaps the remaining Wo matmuls.
- LN2 apply split per-chunk so the first W1 matmul (contraction chunk 0)
  starts without waiting for the whole-tile apply.
- Output stored bf16 ([P,FC,TQ] contiguous, half the DMA) and converted to
  f32 on host.

Host-side folds (f32): Wq_eff = diag(ln1_w) Wq, bq_eff = ln1_b@Wq + bq (same
k/v); bo_eff = (ln1_b@Wv + bv)@Wo + bo; W1_eff = diag(ln2_w) W1,
b1_eff = ln2_b@W1 + b1.  fp8 weights are scaled by 64 before the e4m3 cast
(undone at PSUM evacuation); K additionally folds the 1/32 score scale
(net x256 into fp8, 1/8192 out).
"""

import os
import numpy as np
import ml_dtypes

DEBUG_TAPS = bool(int(os.environ.get("KDBG", "0")))

import concourse.bass as bass
import concourse.tile as tile
from concourse import bacc, mybir
from concourse.bass import ts, ds
from concourse.alu_op_type import AluOpType
from concourse.bass_utils import run_bass_kernel_spmd

F32 = mybir.dt.float32
BF16 = mybir.dt.bfloat16
FP8 = mybir.dt.float8e4
AF = mybir.ActivationFunctionType
DR = mybir.MatmulPerfMode.DoubleRow

H = 768
FF = 3072
NH = 12
DH = 64
B = 2
S = 2048
P = 128
NCORES = 8
TQ = 512          # own tokens per core
NTT = S // TQ     # 4 token tiles per batch
FC = H // P       # 6 feature chunks
FCP = FC // 2     # 3 feature chunk pairs
FFC = FF // P     # 24 hidden chunks
FFCP = FFC // 2   # 12 hidden chunk pairs
TKC = S // P      # 16 key token chunks
HPAIRS = NH // 2  # 6 head pairs
EPS = 1e-6
WS = 64.0         # fp8 weight pre-scale
RWS = 1.0 / WS
KSC = 1.0 / 32.0  # score pre-scale folded into K (psum u = score/32)
WS_K = 8192.0     # fp8 pre-scale for the K weights (covers the 1/32 fold)
RWS_K = 1.0 / WS_K

# exp(4u) ~= (1 + EC1 u + EC2 u^2 + EC3 u^3)^4, u in [-0.72, 0.64]
EC1 = 1.003814416855575
EC2 = 0.5145265913015553
EC3 = 0.15513701271984487


def _register_exp4():
    from concourse import dve_ops
    from concourse.dve_ops import DveOp, OPS, CUSTOM_DVE_SPECS
    from concourse.dve_spec import Spec, Src0, C0, C1, C2, One, sq, lower
    from concourse.dve_uop import DveOpSpec

    name = "EXP4_ANT"
    for op in OPS:
        if op.name == name:
            return op
    u = Src0
    body = sq(sq(((u * C2 + C1) * u + C0) * u + One))

    def _ref(in0, s0, s1, imm2):
        p = 1.0 + s0 * in0 + s1 * in0 * in0 + imm2 * in0 * in0 * in0
        return (p * p * (p * p)).astype(np.float32)

    spec = Spec(body=body, reference=_ref)
    row = dve_ops._CUSTOM_DVE_ROW_BASE + len(OPS)
    assert row < 0x20
    dve_ops._SUB_OPCODE_FOR_NAME[name] = row
    shas = {}
    for ver in ("v3", "v4"):
        s = DveOpSpec(name=name, opcode=row, uops=lower(spec, ver=ver),
                      rd1_en=False)
        shas[ver] = s.sha(ver)
    op = DveOp(name, spec, subdim=False, uops_sha=shas)
    OPS.append(op)
    CUSTOM_DVE_SPECS[name] = spec
    return op


EXP4_OP = _register_exp4()


def build():
    nc = bacc.Bacc("TRN2", target_bir_lowering=False, debug=False,
                   num_devices=NCORES)

    latq_d = nc.dram_tensor("latTq", [P, FC, TQ], F32, kind="ExternalInput")
    latbf_d = nc.dram_tensor("latTbf", [NTT, P, FC, TQ], BF16,
                             kind="ExternalInput")
    wq_d = nc.dram_tensor("wq", [P, FC, H], FP8, kind="ExternalInput")
    wk_d = nc.dram_tensor("wk", [P, FC, H], FP8, kind="ExternalInput")
    wv_d = nc.dram_tensor("wv", [P, FC, H], FP8, kind="ExternalInput")
    wo_d = nc.dram_tensor("wo", [DH, HPAIRS, 2, H], FP8, kind="ExternalInput")
    w1_d = nc.dram_tensor("w1", [FFC, P, FC, P], BF16, kind="ExternalInput")
    w2_d = nc.dram_tensor("w2", [P, FFCP, 2, H], FP8, kind="ExternalInput")
    bq_d = nc.dram_tensor("bq", [P, FC], F32, kind="ExternalInput")
    bk_d = nc.dram_tensor("bk", [P, FC], F32, kind="ExternalInput")
    bo_d = nc.dram_tensor("bo", [P, FC], F32, kind="ExternalInput")
    b1_d = nc.dram_tensor("b1", [P, FFC], F32, kind="ExternalInput")
    b2_d = nc.dram_tensor("b2", [P, FC], F32, kind="ExternalInput")
    out_d = nc.dram_tensor("outT", [P, FC, TQ], BF16, kind="ExternalOutput")
    if DEBUG_TAPS:
        dbg_lat2_d = nc.dram_tensor("dbg_lat2", [P, FC, TQ], F32,
                                    kind="ExternalOutput")
        dbg_nx2_d = nc.dram_tensor("dbg_nx2", [P, FC, TQ], BF16,
                                   kind="ExternalOutput")
        dbg_ctx_d = nc.dram_tensor("dbg_ctx", [DH, 2, TQ], FP8,
                                   kind="ExternalOutput")
        dbg_k_d = nc.dram_tensor("dbg_k", [P, FC, TQ], BF16,
                                 kind="ExternalOutput")

    with tile.TileContext(nc) as tc:
        with (
            tc.tile_pool(name="consts", bufs=1) as consts,
            tc.tile_pool(name="persist", bufs=1) as persist,
        ):
            # constants (vector ring for the small bias DMAs)
            ones_col_bf = consts.tile([P, 1], BF16)
            nc.vector.memset(ones_col_bf[:], 1.0)
            ones_row_bf = consts.tile([1, P], BF16)
            nc.vector.memset(ones_row_bf[:], 1.0)
            eps_tile = consts.tile([1, 1], F32)
            nc.vector.memset(eps_tile[:], EPS)
            zero_col = consts.tile([P, 1], F32)
            nc.vector.memset(zero_col[:], 0.0)
            bq_sb = consts.tile([P, FC], F32)
            nc.gpsimd.dma_start(bq_sb[:], bq_d.ap())
            bk_sb = consts.tile([P, FC], F32)
            nc.gpsimd.dma_start(bk_sb[:], bk_d.ap())
            bo_sb = consts.tile([P, FC], F32)
            nc.gpsimd.dma_start(bo_sb[:], bo_d.ap())
            b1_sb = consts.tile([P, FFC], F32)
            nc.gpsimd.dma_start(b1_sb[:], b1_d.ap())
            b2_sb = consts.tile([P, FC], F32)
            nc.gpsimd.dma_start(b2_sb[:], b2_d.ap())

            # persistent activations
            kT = [persist.tile([P, FC, TQ], BF16, tag=f"kT{t}",
                               name=f"kT{t}")
                  for t in range(NTT)]
            v_sb = persist.tile([P, TKC, NH, DH + 1], BF16)
            nc.vector.memset(v_sb[:, :, :, DH:DH + 1], 1.0)
            qT = persist.tile([P, FC, TQ], BF16)
            ctxP = [persist.tile([DH, 2, TQ], FP8, tag=f"ctxP{hp}",
                                 name=f"ctxP{hp}")
                    for hp in range(HPAIRS)]
            resid1 = persist.tile([P, FC, TQ], F32, tag="bigf32")
            lat2T = persist.tile([P, FC, TQ], BF16, tag="lat2")
            nx2 = persist.tile([P, FC, TQ], BF16, tag="nx2")

            wo_sb = persist.tile([DH, HPAIRS, 2, H], FP8, tag="wo")

            def ln_tail(T, ps_sum, ps_sq, small_pool, ab_pool, bcast_mm=None):
                """sum/sqsum rows -> broadcast alpha/beta [P,T] tiles.

                bcast_mm=(psum_pool, tagA, tagB): broadcast via K=1 matmuls
                into PSUM (~0.35us on the idle PE) instead of ~1.3us gpsimd
                partition_broadcasts — used on the LN2 critical path."""
                mu = small_pool.tile([1, T], F32, tag="lnsmall")
                nc.scalar.mul(mu[:], ps_sum, 1.0 / H)
                mu2 = small_pool.tile([1, T], F32, tag="lnsmall")
                nc.vector.tensor_mul(mu2[:], mu[:], mu[:])
                msq = small_pool.tile([1, T], F32, tag="lnsmall")
                nc.scalar.mul(msq[:], ps_sq, 1.0 / H)
                var = small_pool.tile([1, T], F32, tag="lnsmall")
                nc.vector.tensor_sub(var[:], msq[:], mu2[:])
                sd = small_pool.tile([1, T], F32, tag="lnsmall")
                nc.scalar.activation(sd[:], var[:], AF.Sqrt, bias=eps_tile[:])
                rsig = small_pool.tile([1, T], F32, tag="lnsmall")
                nc.vector.reciprocal_approx_fast(rsig[:], sd[:])
                rsig_bf = small_pool.tile([1, T], BF16, tag="lnsmallbf")
                nc.scalar.copy(rsig_bf[:], rsig[:])
                beta_bf = small_pool.tile([1, T], BF16, tag="lnsmallbf")
                nc.vector.scalar_tensor_tensor(beta_bf[:], mu[:], -1.0, rsig[:],
                                               AluOpType.mult, AluOpType.mult)
                if bcast_mm is not None:
                    pool, tagA, tagB = bcast_mm
                    ab = pool.tile([P, T], F32, tag=tagA, name="ab_mm")
                    nc.tensor.matmul(ab[:], ones_row_bf[:], rsig_bf[:],
                                     start=True, stop=True)
                    bb = pool.tile([P, T], F32, tag=tagB, name="bb_mm")
                    nc.tensor.matmul(bb[:], ones_row_bf[:], beta_bf[:],
                                     start=True, stop=True)
                    return ab, bb
                ab = ab_pool.tile([P, T], BF16, tag="ab")
                nc.gpsimd.partition_broadcast(ab[:], rsig_bf[0:1, :])
                bb = ab_pool.tile([P, T], BF16, tag="bb")
                nc.gpsimd.partition_broadcast(bb[:], beta_bf[0:1, :])
                return ab, bb

            # ---------------- Phase 1: LN1 + K/V/Q projections ----------------
            with (
                tc.tile_pool(name="wproj", bufs=1) as wproj,
                tc.tile_pool(name="latp", bufs=4) as latp,
                tc.tile_pool(name="sqp", bufs=2) as sqp,
                tc.tile_pool(name="lntp", bufs=2) as lntp,
                tc.tile_pool(name="nxp", bufs=4) as nxp,
                tc.tile_pool(name="abp", bufs=2) as abp,
                tc.tile_pool(name="smallp", bufs=6) as smallp,
                tc.tile_pool(name="ps_stats", bufs=4, space="PSUM") as ps_stats,
                tc.tile_pool(name="ps_kq", bufs=2, space="PSUM") as ps_kq,
                tc.tile_pool(name="ps_v", bufs=1, space="PSUM") as ps_v,
            ):
                # activations first on the sync ring (needed before any MM),
                # weights behind them on the scalar ring in need-order
                # one ring, strict need order: concurrent queues share the
                # same 16 DMA engines, so a second ring only steals bandwidth
                latbf_tiles = [latp.tile([P, FC, TQ], BF16, tag="latbf",
                                         name=f"latbf{t}")
                               for t in range(NTT)]
                nc.sync.dma_start(latbf_tiles[0][:], latbf_d.ap()[0])
                wk_sb = wproj.tile([P, FC, H], FP8, tag="wk")
                nc.sync.dma_start(wk_sb[:], wk_d.ap())
                nc.sync.dma_start(latbf_tiles[1][:], latbf_d.ap()[1])
                wv_sb = wproj.tile([P, FC, H], FP8, tag="wv")
                nc.sync.dma_start(wv_sb[:], wv_d.ap())
                nc.sync.dma_start(latbf_tiles[2][:], latbf_d.ap()[2])
                nc.sync.dma_start(latbf_tiles[3][:], latbf_d.ap()[3])
                wq_sb = wproj.tile([P, FC, H], FP8, tag="wq")
                nc.sync.dma_start(wq_sb[:], wq_d.ap())

                # ~6us of dep-free matmuls during the initial DMA dead
                # window: trips the HAM SHORT window so real work starts at
                # 2.4GHz instead of paying the cold-clock ramp repeatedly
                warm_src = latp.tile([P, TQ], BF16, tag="warmsrc")
                nc.vector.memset(warm_src[:], 0.0)
                warm_ps = ps_stats.tile([1, TQ], F32, tag="stats",
                                        name="warmps")
                for w in range(16):
                    nc.tensor.matmul(warm_ps[0:1, :], ones_col_bf[:],
                                     warm_src[:], start=(w == 0),
                                     stop=(w == 15))

                def emit_stats(tt):
                    latbf_t = latbf_tiles[tt]
                    sq_t = sqp.tile([P, FC, TQ], BF16, tag="sq",
                                    name=f"sq{tt}")
                    nc.scalar.activation(sq_t[:], latbf_t[:], AF.Square)
                    ps_stat = ps_stats.tile([33, TQ], F32, tag="stats",
                                            name=f"stat{tt}")
                    for c in range(FC):
                        nc.tensor.matmul(ps_stat[0:1, :], ones_col_bf[:],
                                         latbf_t[:, c, :],
                                         start=(c == 0), stop=(c == FC - 1))
                    for c in range(FC):
                        nc.tensor.matmul(ps_stat[32:33, :], ones_col_bf[:],
                                         sq_t[:, c, :],
                                         start=(c == 0), stop=(c == FC - 1))
                    return latbf_t, ps_stat

                # interleaved emission: tail(t) lands on ACT before the next
                # tile's Square so the LN chain isn't stuck behind 3 queued
                # whole-tile Squares
                pend = [emit_stats(0)]
                for tt in range(NTT):
                    latbf_t, ps_stat = pend[tt]
                    ab, bb = ln_tail(TQ, ps_stat[0:1, :], ps_stat[32:33, :],
                                     smallp, abp)
                    if tt + 1 < NTT:
                        pend.append(emit_stats(tt + 1))
                    nx_t = nxp.tile([P, FC, TQ], FP8, tag="nx")
                    # fused whole-tile LN apply (stride-0 chunk broadcast)
                    t = lntp.tile([P, FC, TQ], BF16, tag="lnt",
                                  name=f"lnt{tt}")
                    nc.vector.tensor_mul(
                        t[:], latbf_t[:],
                        ab[:].unsqueeze(1).broadcast_to((P, FC, TQ)))
                    nc.vector.tensor_add(
                        nx_t[:], t[:],
                        bb[:].unsqueeze(1).broadcast_to((P, FC, TQ)))

                    # K projection (feature-major out, 1/32 score scale folded)
                    for mc in range(FC):
                        ps = ps_kq.tile([P, TQ], F32, tag="kq")
                        for cp in range(FCP):
                            nc.tensor.matmul(ps[:],
                                             wk_sb[:, 2 * cp:2 * cp + 2, ts(mc, P)],
                                             nx_t[:, 2 * cp:2 * cp + 2, :],
                                             start=(cp == 0), stop=(cp == FCP - 1),
                                             perf_mode=DR)
                        nc.scalar.activation(kT[tt][:, mc, :], ps[:],
                                             AF.Identity, bias=bk_sb[:, mc:mc + 1],
                                             scale=RWS_K)
                    # V projection (token-major out, ones col preset); two
                    # bank-aligned 384-wide psum groups, single evacuation
                    for tcl in range(TQ // P):
                        tcg = tt * (TQ // P) + tcl
                        ps = ps_v.tile([P, 2, TQ], F32, tag="v")
                        for half in range(2):
                            for cp in range(FCP):
                                nc.tensor.matmul(
                                    ps[:, half, 0:384],
                                    nx_t[:, 2 * cp:2 * cp + 2, ts(tcl, P)],
                                    wv_sb[:, 2 * cp:2 * cp + 2,
                                          ds(half * 384, 384)],
                                    start=(cp == 0), stop=(cp == FCP - 1),
                                    perf_mode=DR)
                        nc.vector.tensor_scalar_mul(
                            v_sb[:, tcg, :, 0:DH].rearrange(
                                "p (two h) d -> p two h d", two=2),
                            ps[:, :, 0:384].rearrange(
                                "p two (h d) -> p two h d", d=DH),
                            RWS)
                    if tt == 0:
                        nx0 = nx_t

                # Q projection last: wq is the last DMA, and attention's
                # ctx matmuls gate on the V psum banks releasing -- pulling
                # all latbf tiles + V earlier shortens the attention lead-in
                for mc in range(FC):
                    ps = ps_kq.tile([P, TQ], F32, tag="kq")
                    for cp in range(FCP):
                        nc.tensor.matmul(
                            ps[:],
                            wq_sb[:, 2 * cp:2 * cp + 2, ts(mc, P)],
                            nx0[:, 2 * cp:2 * cp + 2, :],
                            start=(cp == 0), stop=(cp == FCP - 1),
                            perf_mode=DR)
                    nc.scalar.activation(qT[:, mc, :], ps[:],
                                         AF.Identity,
                                         bias=bq_sb[:, mc:mc + 1],
                                         scale=RWS)

            # ------------- Phase 2+3: attention, Wo+LN2, FFN -------------
            with (
                tc.tile_pool(name="wffn", bufs=1) as wffn,
                tc.tile_pool(name="attnp", bufs=8) as attnp,
                tc.tile_pool(name="w1sp", bufs=6) as w1sp,
                tc.tile_pool(name="rrow", bufs=2) as rrow,
                tc.tile_pool(name="rbp", bufs=2) as rbp,
                tc.tile_pool(name="sq2p", bufs=1) as sq2p,
                tc.tile_pool(name="ab2p", bufs=1) as ab2p,
                tc.tile_pool(name="small2p", bufs=5) as small2p,
                tc.tile_pool(name="hp_pool", bufs=2) as hp_pool,
            ):
                # deferred input DMAs: wo (scalar), resid1 (gpsimd), w2 (scalar)
                nc.scalar.dma_start(wo_sb[:], wo_d.ap())
                nc.gpsimd.dma_start(resid1[:], latq_d.ap())
                w2_sb = wffn.tile([P, FFCP, 2, H], FP8, tag="w2")
                nc.scalar.dma_start(w2_sb[:], w2_d.ap())
                with (
                    tc.tile_pool(name="ps_sc", bufs=2, space="PSUM") as ps_sc,
                    tc.tile_pool(name="ps_ctx", bufs=1, space="PSUM") as ps_ctx,
                ):
                    for hps in [(0, 1), (2, 3), (4, 5)]:
                        ctx_tiles = {}
                        for hp in hps:
                            ctxA_ps = ps_ctx.tile([DH + 1, TQ], F32,
                                                  tag=f"ctxA{hp % 2}",
                                                  name=f"ctxA{hp}")
                            ctxB_ps = ps_ctx.tile([DH + 1, TQ], F32,
                                                  tag=f"ctxB{hp % 2}",
                                                  name=f"ctxB{hp}")
                            ctx_tiles[hp] = (ctxA_ps, ctxB_ps)
                        # software-pipelined by one unit: ctx(k-1) is
                        # emitted AFTER scores(k)+exp(k), so the in-order
                        # tensor queue has ready score work in front of the
                        # exp-gated ctx matmuls instead of stalling ~1.2us
                        # per unit.  Each unit's exp runs as two half ops on
                        # ONE engine (units alternate engines: whole-unit
                        # assignment keeps the engines' sem chains off each
                        # other), so ctxA waits only the first half.
                        def emit_ctxA(pend):
                            pj, phA, phB, pa2, pA_ps, pB_ps = pend
                            nc.tensor.matmul(pA_ps[:], v_sb[:, pj, phA, :],
                                             pa2[:, 0, :],
                                             start=(pj == 0),
                                             stop=(pj == TKC - 1))

                        def emit_ctxB(pend):
                            pj, phA, phB, pa2, pA_ps, pB_ps = pend
                            nc.tensor.matmul(pB_ps[:], v_sb[:, pj, phB, :],
                                             pa2[:, 1, :],
                                             start=(pj == 0),
                                             stop=(pj == TKC - 1))

                        ui = 0
                        pend1 = None  # awaiting ctxA (1 unit behind)
                        pend2 = None  # awaiting ctxB (2 units behind)
                        for j in range(TKC):
                            jt, jj = j // (TQ // P), j % (TQ // P)
                            for hp in hps:
                                hA, hB = 2 * hp, 2 * hp + 1
                                scA = ps_sc.tile([P, TQ], F32, tag="scA")
                                scB = ps_sc.tile([P, TQ], F32, tag="scB")
                                nc.tensor.matmul(scA[:],
                                                 kT[jt][0:DH, hp, ts(jj, P)],
                                                 qT[0:DH, hp, :],
                                                 start=True, stop=True)
                                nc.tensor.matmul(scB[:],
                                                 kT[jt][DH:P, hp, ts(jj, P)],
                                                 qT[DH:P, hp, :],
                                                 start=True, stop=True)
                                a2 = attnp.tile([P, 2, TQ], FP8, tag="attn")
                                if ui % 2 == 1:
                                    nc.vector._custom_dve(
                                        EXP4_OP, out=a2[:, 0, :],
                                        in0=scA[:],
                                        s0=EC1, s1=EC2, imm2=EC3)
                                    nc.vector._custom_dve(
                                        EXP4_OP, out=a2[:, 1, :],
                                        in0=scB[:],
                                        s0=EC1, s1=EC2, imm2=EC3)
                                else:
                                    nc.scalar.activation(a2[:, 0, :],
                                                         scA[:],
                                                         AF.Exp, scale=4.0,
                                                         bias=zero_col[:])
                                    nc.scalar.activation(a2[:, 1, :],
                                                         scB[:],
                                                         AF.Exp, scale=4.0,
                                                         bias=zero_col[:])
                                ui += 1
                                if pend1 is not None:
                                    emit_ctxA(pend1)
                                if pend2 is not None:
                                    emit_ctxB(pend2)
                                ctxA_ps, ctxB_ps = ctx_tiles[hp]
                                pend2 = pend1
                                pend1 = (j, hA, hB, a2, ctxA_ps, ctxB_ps)
                        emit_ctxA(pend1)
                        emit_ctxB(pend2)
                        emit_ctxB(pend1)
                        for hp in hps:
                            ctxA_ps, ctxB_ps = ctx_tiles[hp]
                            # ACT copy shifts the denom row to partition 0
                            # (custom DVE ops require offset-0 operands)
                            dA = rrow.tile([1, TQ], F32, tag="dr")
                            nc.scalar.copy(dA[:], ctxA_ps[DH:DH + 1, :])
                            dB = rrow.tile([1, TQ], F32, tag="dr")
                            nc.scalar.copy(dB[:], ctxB_ps[DH:DH + 1, :])
                            rA = rrow.tile([1, TQ], F32, tag="rr")
                            nc.vector.reciprocal_approx_fast(rA[:], dA[:])
                            rB = rrow.tile([1, TQ], F32, tag="rr")
                            nc.vector.reciprocal_approx_fast(rB[:], dB[:])
                            rb = rbp.tile([DH, 2, TQ], F32, tag="rb")
                            nc.gpsimd.partition_broadcast(rb[:, 0, :], rA[0:1, :])
                            nc.gpsimd.partition_broadcast(rb[:, 1, :], rB[0:1, :])
                            # x16 lifts ctx out of e4m3's denormal range
                            # (undone in the Wo evacuation scale)
                            nc.vector.scalar_tensor_tensor(
                                ctxP[hp][:, 0, :], ctxA_ps[0:DH, :], 16.0,
                                rb[:, 0, :], AluOpType.mult, AluOpType.mult)
                            nc.vector.scalar_tensor_tensor(
                                ctxP[hp][:, 1, :], ctxB_ps[0:DH, :], 16.0,
                                rb[:, 1, :], AluOpType.mult, AluOpType.mult)

                    # ---- Wo (DoubleRow over head pairs) + residual + LN2 ----
                    # mc-outer: each wo psum chunk completes after 6 MMs and
                    # its residual/stats chain overlaps the remaining chunks
                    wo_tags = ["ctxA0", "ctxB0", "ctxA1", "ctxB1", "scA", "scB"]
                    wo_ps = []
                    for mc in range(FC):
                        pool = ps_ctx if mc < 4 else ps_sc
                        ps = pool.tile([P, TQ], F32, tag=wo_tags[mc],
                                       name=f"wops{mc}")
                        wo_ps.append(ps[:])
                    # stats psums alias wo_ps[0]/wo_ps[1]; their first writes
                    # serialize behind those chunks' affine evacuations
                    ps_sum2 = ps_ctx.tile([1, TQ], F32, tag="ctxA0")
                    ps_sq2 = ps_ctx.tile([33, TQ], F32, tag="ctxB0")
                    for mc in range(FC):
                        for hp in range(HPAIRS):
                            nc.tensor.matmul(wo_ps[mc],
                                             wo_sb[:, hp, :, ts(mc, P)],
                                             ctxP[hp][:],
                                             start=(hp == 0),
                                             stop=(hp == HPAIRS - 1),
                                             perf_mode=DR)
                    for mc in range(FC):
                        # lat2 written bf16 directly: no separate copy; the
                        # residual/stats/LN2 consumers all take bf16
                        nc.vector.affine_then_add(lat2T[:, mc, :], wo_ps[mc],
                                                  resid1[:, mc, :], RWS / 16.0,
                                                  bo_sb[:, mc:mc + 1])
                        sq2 = sq2p.tile([P, TQ], BF16, tag="sq2",
                                        name=f"sq2_{mc}")
                        nc.scalar.activation(sq2[:], lat2T[:, mc, :],
                                             AF.Square)
                        nc.tensor.matmul(ps_sum2[0:1, :], ones_col_bf[:],
                                         lat2T[:, mc, :],
                                         start=(mc == 0), stop=(mc == FC - 1))
                        nc.tensor.matmul(ps_sq2[32:33, :], ones_col_bf[:],
                                         sq2[:],
                                         start=(mc == 0), stop=(mc == FC - 1))
                    ab2, bb2 = ln_tail(TQ, ps_sum2[0:1, :], ps_sq2[32:33, :],
                                       small2p, ab2p,
                                       bcast_mm=(ps_sc, "scA", "scB"))
                    # per-chunk LN2 apply so W1's first contraction chunk
                    # starts before the whole tile is normalized
                    for mc in range(FC):
                        t2c = sq2p.tile([P, TQ], BF16, tag="lnt2",
                                        name=f"lnt2_{mc}")
                        nc.vector.tensor_mul(t2c[:], lat2T[:, mc, :], ab2[:])
                        nc.vector.tensor_add(nx2[:, mc, :], t2c[:], bb2[:])

                if DEBUG_TAPS:
                    nc.sync.dma_start(dbg_lat2_d.ap(), lat2T[:])
                    nc.sync.dma_start(dbg_nx2_d.ap(), nx2[:])
                    nc.sync.dma_start(dbg_ctx_d.ap(), ctxP[0][:])
                    nc.sync.dma_start(dbg_k_d.ap(), kT[0][:])

                # ---- FFN ----
                outT = persist.tile([P, FC, TQ], BF16, tag="outbf")
                with (
                    tc.tile_pool(name="ps_fo", bufs=1, space="PSUM") as ps_fo,
                    tc.tile_pool(name="ps_h", bufs=2, space="PSUM") as ps_h,
                ):
                    ps_out = ps_fo.tile([P, FC, TQ], F32, tag="fo")
                    for mhp in range(FFCP):
                        h2 = hp_pool.tile([P, 2, TQ], FP8, tag="h2")
                        for i in range(2):
                            mh = 2 * mhp + i
                            w1t = w1sp.tile([P, FC, P], BF16, tag="w1s",
                                            name=f"w1t{mh}")
                            nc.sync.dma_start(w1t[:], w1_d.ap()[mh])
                            # one [P,TQ] psum (= a full bank) per mh: two
                            # accumulation groups may not share a 2KB bank
                            psh = ps_h.tile([P, TQ], F32, tag="h",
                                            name=f"psh{mh}")
                            for kc in range(FC):
                                nc.tensor.matmul(psh[:],
                                                 w1t[:, kc, :],
                                                 nx2[:, kc, :],
                                                 start=(kc == 0),
                                                 stop=(kc == FC - 1))
                            nc.scalar.activation(h2[:, i, :], psh[:],
                                                 AF.Gelu,
                                                 bias=b1_sb[:, mh:mh + 1])
                        for mc in range(FC):
                            nc.tensor.matmul(ps_out[:, mc, :],
                                             w2_sb[:, mhp, :, ts(mc, P)],
                                             h2[:],
                                             start=(mhp == 0),
                                             stop=(mhp == FFCP - 1),
                                             perf_mode=DR)
                    for mc in range(FC):
                        nc.vector.affine_then_add(outT[:, mc, :],
                                                  ps_out[:, mc, :],
                                                  lat2T[:, mc, :], RWS,
                                                  b2_sb[:, mc:mc + 1])
                        nc.sync.dma_start(out_d.ap()[:, mc, :], outT[:, mc, :])

    nc.compile()
    return nc


_NC_CACHE = {}


def _get_nc():
    if "nc" not in _NC_CACHE:
        _NC_CACHE["nc"] = build()
    return _NC_CACHE["nc"]


def _to_fp8(w, scale=WS):
    return (np.asarray(w, np.float32) * scale).astype(ml_dtypes.float8_e4m3)


def _prep_inputs(latent, ln1_w, ln1_b, Wq, bq, Wk, bk, Wv, bv, Wo, bo,
                 ln2_w, ln2_b, W1, b1, W2, b2):
    f32 = np.float32
    bf16 = ml_dtypes.bfloat16
    lat = np.asarray(latent, f32)
    ln1_w = np.asarray(ln1_w, f32); ln1_b = np.asarray(ln1_b, f32)
    ln2_w = np.asarray(ln2_w, f32); ln2_b = np.asarray(ln2_b, f32)
    Wq = np.asarray(Wq, f32); Wk = np.asarray(Wk, f32); Wv = np.asarray(Wv, f32)
    Wo = np.asarray(Wo, f32); W1 = np.asarray(W1, f32); W2 = np.asarray(W2, f32)
    bq = np.asarray(bq, f32); bk = np.asarray(bk, f32); bv = np.asarray(bv, f32)
    bo = np.asarray(bo, f32); b1 = np.asarray(b1, f32); b2 = np.asarray(b2, f32)

    wq_eff = ln1_w[:, None] * Wq
    wk_eff = ln1_w[:, None] * Wk * KSC   # 1/32 score scale folded into K
    wv_eff = ln1_w[:, None] * Wv
    bq_eff = ln1_b @ Wq + bq
    bk_eff = (ln1_b @ Wk + bk) * KSC
    bv_eff = ln1_b @ Wv + bv
    bo_eff = bv_eff @ Wo + bo
    w1_eff = ln2_w[:, None] * W1
    b1_eff = ln2_b @ W1 + b1

    def chunk_in(w):  # [H, M] -> [P, FC, M]  (contraction chunked)
        return np.ascontiguousarray(
            w.reshape(FC, P, -1).transpose(1, 0, 2))

    wq8 = _to_fp8(chunk_in(wq_eff))
    wk8 = _to_fp8(chunk_in(wk_eff), WS_K)
    wv8 = _to_fp8(chunk_in(wv_eff))
    # Wo: [H, H] -> [DH, HPAIRS, 2, H]  rows hp*128 + i*64 + p
    wo8 = _to_fp8(np.ascontiguousarray(
        Wo.reshape(HPAIRS, 2, DH, H).transpose(2, 0, 1, 3)))
    # W1: [H, FF] -> [FFC, P, FC, P] bf16 (mh-major for streamed tiles)
    w1b = np.ascontiguousarray(
        chunk_in(w1_eff).reshape(P, FC, FFC, P).transpose(2, 0, 1, 3)
    ).astype(ml_dtypes.bfloat16)
    # W2: [FF, H] -> [P, FFCP, 2, H]  rows (2j+i)*128 + p, fp8 x64
    w28 = _to_fp8(np.ascontiguousarray(
        W2.reshape(FFCP, 2, P, H).transpose(2, 0, 1, 3)))

    def chunked(b):  # [H or FF] -> [P, nchunks]
        return np.ascontiguousarray(b.reshape(-1, P).T)

    common = {
        "wq": wq8, "wk": wk8, "wv": wv8, "wo": wo8,
        "w1": w1b, "w2": w28,
        "bq": chunked(bq_eff), "bk": chunked(bk_eff), "bo": chunked(bo_eff),
        "b1": chunked(b1_eff), "b2": chunked(b2),
    }
    in_maps = []
    for c in range(NCORES):
        b = c // (NCORES // B)
        q = c % (NCORES // B)
        latT_c = np.ascontiguousarray(np.roll(lat[b].T, -q * TQ, axis=1))
        m = dict(common)
        # [P, FC, TQ] contiguous (12KB per-partition DMA lines)
        m["latTq"] = np.ascontiguousarray(
            latT_c[:, :TQ].reshape(FC, P, TQ).transpose(1, 0, 2))
        # [NTT, P, FC, TQ] contiguous per tile (6KB lines)
        m["latTbf"] = np.ascontiguousarray(
            latT_c.astype(bf16).reshape(FC, P, NTT, TQ).transpose(2, 1, 0, 3))
        in_maps.append(m)
    return in_maps


def _unshard(res):
    out = np.empty((B, S, H), np.float32)
    for c in range(NCORES):
        b = c // (NCORES // B)
        q = c % (NCORES // B)
        o = np.asarray(res.results[c]["outT"], dtype=np.float32)  # [P, FC, TQ]
        out[b, q * TQ:(q + 1) * TQ, :] = o.transpose(1, 0, 2).reshape(H, TQ).T
    return out


def kernel(**inputs):
    nc = _get_nc()
    in_maps = _prep_inputs(**inputs)
    res = run_bass_kernel_spmd(nc, in_maps, core_ids=list(range(NCORES)))
    return _unshard(res)
